# revision 27
# baseline (speedup 1.0000x reference)
"""Trainium2 Bass kernel for GQA causal sliding-window self-attention.

Sharding: 8 cores = 2 (batch) x 4 (KV-head groups). Each core handles one
batch element and one KV head with its 3 GQA query heads. The output
projection is computed per-group against the matching Wproj column slice;
the 4 partial outputs per batch are summed on the host.

v2 design (vs the fp32r baseline):
  * all matmul operands are bf16 (full PE rate at ANY moving width, half
    the DMA bytes, 2-4x DVE throughput); PSUM accumulation stays fp32.
  * single software pipeline over t-chunks: projections+rope for chunk i,
    then attention for chunk i with the out-projection of chunk i-1
    interleaved into the s-tile loop as PE filler.
  * attention matmuls only cover the valid (causal+window) column range of
    each 128-row s-tile; triangle masks are added with constant 128x128
    -1e9 blocks via identity matmuls.
  * row-sum reductions (softmax denominator, rmsnorm sum-of-squares, the
    ve gate) are computed as tiny 1-column matmuls (lhsT = data tile) so
    they cost ~nothing on the PE.
  * the k-side rmsnorm scale is folded into the exp's per-partition scale
    operand; the global 1.2*1.2/sqrt(D) score scale is folded into the
    cos/sin tables (sqrt of it on each of q and k).
  * v is projected directly in natural [s, D] layout (no transposes), the
    gate applied via a per-partition tensor_scalar multiply.
"""

import os
import sys
import numpy as np

sys.path.insert(0, "/opt/trn_rl_repo")

from contextlib import ExitStack

import ml_dtypes

from concourse import mybir, bacc, tile
from concourse.bass_utils import run_bass_kernel_spmd

f32 = mybir.dt.float32
f32r = mybir.dt.float32r
bf16 = mybir.dt.bfloat16
AF = mybir.ActivationFunctionType

B, T, C = 2, 2048, 1536
H, KV, D = 12, 4, 128
REP = H // KV          # 3 query heads per kv head
QD = REP * D           # 384
VE_GATE_CH = 12
N_CORES = 8
TC = 512               # t-chunk width
HTC = TC // 2
NTC = T // TC          # 4
NCC = C // 128         # 12 contraction chunks
NST = T // 128         # 16 s-tiles
NSUB = TC // 128       # 4 col-subtiles per t-chunk

_EPS = float(np.finfo(np.float32).eps)
# sqrt(1.2*1.2/sqrt(D)) folded into both cos/sin tables (q and k sides)
_CS = float(np.sqrt(1.2 * 1.2 / np.sqrt(D)))
_MASKVAL = -1.0e9

_CACHE = {}


def _setup_act_tables():
    """Reorder activation-table sets so ln+exp share one set (avoids ~33
    table reloads).  Patches both the bacc-side set picker and the walrus
    --act-root-json (they must agree on set indices)."""
    try:
        import json
        import tempfile
        import concourse.hw_specs as hw_specs
        import concourse.bacc as bacc_mod
        from neuronxcc.driver.Job import Job
        from neuronxcc.driver.jobs.support.FindActInfo import findActInfoFile

        src = findActInfoFile(Job.getPackageDir(), "gen3")
        if not src or not os.path.exists(src):
            return
        src_dir = os.path.dirname(src)
        dst = os.path.join(tempfile.gettempdir(), "bass_act_pwp_lnexp")
        os.makedirs(dst, exist_ok=True)
        for f in os.listdir(src_dir):
            tgt = os.path.join(dst, f)
            if not os.path.exists(tgt):
                try:
                    os.symlink(os.path.join(src_dir, f), tgt)
                except OSError:
                    pass
        d = json.load(open(src))
        sets = d["act_func_sets"]
        idx = [i for i, s in enumerate(sets)
               if s["name"] == "natural_log_exp_and_others"]
        if not idx:
            return
        sets.insert(0, sets.pop(idx[0]))
        jp = os.path.join(dst, "act_info.json")
        if os.path.lexists(jp):
            os.remove(jp)
        json.dump(d, open(jp, "w"))
        os.environ["BASS_ACT_ROOT_JSON_PATH"] = jp

        orig = hw_specs.get_activation_tables

        def reordered(arch):
            t = orig(arch)
            key = "natural_log_exp_and_others"
            if key in t:
                out = {key: t[key]}
                out.update((k, v) for k, v in t.items() if k != key)
                return out
            return t

        hw_specs.get_activation_tables = reordered
        bacc_mod.get_activation_tables = reordered
    except Exception:
        pass


_setup_act_tables()


def _plan_chunk(t0, window, win_finite):
    """Static schedule of the s-tile loop for one t-chunk.

    Returns (r0, recs); recs entries:
      (st, v0, v1, cmask_js, wmask_list, den_ops, yu_pieces)
    where den_ops = [(j, start, stop)], yu_pieces = [(c0, c1, start, stop)],
    wmask_list = [(j, block_idx)] with block 0 = offset r0, 1 = r0-128.
    """
    r0 = ((-window) % 128) if win_finite else 0
    stj0, stj1 = {}, {}
    for j in range(NSUB):
        Tj = t0 + 128 * j
        s_min = max(0, Tj - window) if win_finite else 0
        stj0[j] = s_min // 128
        stj1[j] = Tj // 128
    recs = []
    st_min = min(stj0.values())
    st_max = max(stj1.values())
    for st in range(st_min, st_max + 1):
        cov = [j for j in range(NSUB) if stj0[j] <= st <= stj1[j]]
        if not cov:
            continue
        v0 = 128 * min(cov)
        v1 = 128 * (max(cov) + 1)
        s0 = 128 * st
        cms = [j for j in cov if t0 + 128 * j == s0]
        wms = []
        if win_finite:
            for j in cov:
                off = (t0 + 128 * j) - s0 - window
                if off >= -126:
                    assert off in (r0, r0 - 128), (off, r0)
                    wms.append((j, 0 if off == r0 else 1))
        dens = [(j, False, False) for j in cov]
        recs.append((st, v0, v1, cms, wms, dens, None))
    # PSUM accumulation groups are bank-granular: exactly one start (first
    # matmul; marks the whole bank pending-zero so later first-touches
    # overwrite) and one stop (last matmul) per den/yU group. Each matmul
    # write must be purely-pending or purely-cleared bytes, so split yU
    # pieces at the already-touched high-water mark.
    out = []
    nrec = len(recs)
    hi = 0
    for i, (st, v0, v1, cms, wms, dens, _p) in enumerate(recs):
        dens = [(j, i == 0 and k == 0, i == nrec - 1 and k == len(dens) - 1)
                for k, (j, _f, _l) in enumerate(dens)]
        pieces = []
        if v0 < hi:
            pieces.append((v0, min(v1, hi)))
        if v1 > hi:
            pieces.append((max(v0, hi), v1))
            hi = v1
        pieces = [(c0, c1, i == 0 and k == 0,
                   i == nrec - 1 and k == len(pieces) - 1)
                  for k, (c0, c1) in enumerate(pieces)]
        out.append((st, v0, v1, cms, wms, dens, pieces))
    return r0, out


def _build(window: int):
    win_finite = 0 <= window < T
    plans = [_plan_chunk(tci * TC, window, win_finite) for tci in range(NTC)]

    nc = bacc.Bacc("TRN2", target_bir_lowering=False, debug=False,
                   num_devices=N_CORES)

    xT = nc.dram_tensor("xT", [C, T], bf16, kind="ExternalInput")
    wqT = nc.dram_tensor("wqT", [C, QD], bf16, kind="ExternalInput")
    wkT = nc.dram_tensor("wkT", [C, D], bf16, kind="ExternalInput")
    wvT = nc.dram_tensor("wvT", [C, D], bf16, kind="ExternalInput")
    wpT = nc.dram_tensor("wpT", [QD, C], bf16, kind="ExternalInput")
    wg = nc.dram_tensor("wg", [VE_GATE_CH, 1], bf16, kind="ExternalInput")
    veN = nc.dram_tensor("veN", [T, D], bf16, kind="ExternalInput")
    cos2 = nc.dram_tensor("cos2", [128, T], bf16, kind="ExternalInput")
    sin2m = nc.dram_tensor("sin2m", [128, T], bf16, kind="ExternalInput")
    eye = nc.dram_tensor("eye", [128, 128], bf16, kind="ExternalInput")
    onesI = nc.dram_tensor("onesI", [128, 1], bf16, kind="ExternalInput")
    # mask blocks: [causal, window(off=r0), window(off=r0-128)]
    masksI = nc.dram_tensor("masksI", [3 * 128, 128], bf16,
                            kind="ExternalInput")
    outT = nc.dram_tensor("outT", [C, T], bf16, kind="ExternalOutput")

    xT_re = xT.ap().rearrange("(cc p) t -> p cc t", p=128)

    with tile.TileContext(nc) as tc, ExitStack() as ctx:
        # ---- persistent SBUF pools ----
        pw = ctx.enter_context(tc.tile_pool(name="pw", bufs=1))
        pbig = ctx.enter_context(tc.tile_pool(name="pbig", bufs=1))
        pxt = ctx.enter_context(tc.tile_pool(name="pxt", bufs=5))
        pcs = ctx.enter_context(tc.tile_pool(name="pcs", bufs=2))
        ptmp = ctx.enter_context(tc.tile_pool(name="ptmp", bufs=10))
        prow = ctx.enter_context(tc.tile_pool(name="prow", bufs=4))
        pbc = ctx.enter_context(tc.tile_pool(name="pbc", bufs=2))
        pdbc = ctx.enter_context(tc.tile_pool(name="pdbc", bufs=2))
        pqt = ctx.enter_context(tc.tile_pool(name="pqt", bufs=2))
        pyt = ctx.enter_context(tc.tile_pool(name="pyt", bufs=2))
        pP = ctx.enter_context(tc.tile_pool(name="pP", bufs=5))
        pve = ctx.enter_context(tc.tile_pool(name="pve", bufs=3))
        pout = ctx.enter_context(tc.tile_pool(name="pout", bufs=3))

        # ---- PSUM pools: 8 banks total ----
        psSC = ctx.enter_context(tc.tile_pool(name="psSC", bufs=2,
                                              space="PSUM"))
        psY = ctx.enter_context(tc.tile_pool(name="psY", bufs=2,
                                             space="PSUM"))
        psAcc = ctx.enter_context(tc.tile_pool(name="psAcc", bufs=1,
                                               space="PSUM"))
        psO = ctx.enter_context(tc.tile_pool(name="psO", bufs=2,
                                             space="PSUM"))
        psS = ctx.enter_context(tc.tile_pool(name="psS", bufs=1,
                                             space="PSUM"))

        # one manually-sliced "small" PSUM bank:
        #   cols 0:8    den (4 cols per h, 2-way rotation)
        #   cols 8:16   rmsnorm sum-of-squares (2-way rotation)
        #   cols 16:24  gate (per subtile, 2-way rotation of 4)
        #   cols 128:384  transpose row staging (2 slots of [1,128])
        #   cols 384:512  v natural psum ([128,128])
        small = psS.tile([128, 512], f32, tag="small", name="small")

        # ---- first x chunk + weights; order matters for the startup path:
        # xt(0) half 0 and wk first so the k-stream can start ASAP ----
        wk_sb = pw.tile([128, NCC, D], bf16, tag="wk")
        wq_sb = pw.tile([128, NCC, QD], bf16, tag="wq")
        wqT_re = wqT.ap().rearrange("(cc p) m -> p cc m", p=128)
        wv_sb = pw.tile([128, NCC, D], bf16, tag="wv")
        wvT_re = wvT.ap().rearrange("(cc p) m -> p cc m", p=128)
        wp_sb = pw.tile([128, REP, C], bf16, tag="wp")

        xt0 = []
        for hh in range(2):
            xth = pxt.tile([128, NCC, HTC], bf16, tag="xt", name="xth",
                           bufs=2)
            for g0 in range(0, NCC, 4):
                nc.sync.dma_start(xth[:, g0:g0 + 4, :],
                                  xT_re[:, g0:g0 + 4,
                                        hh * HTC:(hh + 1) * HTC])
            xt0.append(xth)
            if hh == 0:
                nc.scalar.dma_start(
                    wk_sb[:], wkT.ap().rearrange("(cc p) m -> p cc m", p=128))

        wg_sb = pw.tile([VE_GATE_CH, 1], bf16, tag="wg")
        nc.sync.dma_start(wg_sb[:], wg.ap()[:])
        ones_sb = pw.tile([128, 1], bf16, tag="ones")
        nc.sync.dma_start(ones_sb[:], onesI.ap()[:])
        eye_sb = pw.tile([128, 128], bf16, tag="eye")
        nc.scalar.dma_start(eye_sb[:], eye.ap()[:])
        eyeF_sb = pw.tile([128, 128], f32, tag="eyeF")
        nc.scalar.copy(eyeF_sb[:], eye_sb[:])
        masks_sb = pw.tile([128, 3, 128], bf16, tag="masks")
        nc.scalar.dma_start(
            masks_sb[:], masksI.ap().rearrange("(nd p) t -> p nd t", p=128))
        eps_col = pw.tile([128, 1], f32, tag="epsc")
        nc.vector.memset(eps_col[:], _EPS)
        for g0 in range(0, NCC, 6):
            nc.scalar.dma_start(wq_sb[:, g0:g0 + 6, :], wqT_re[:, g0:g0 + 6, :])
        nc.scalar.dma_start(wv_sb[:], wvT_re[:])
        csF = pw.tile([128, T], bf16, tag="csF")
        nc.sync.dma_start(csF[:], cos2.ap()[:])
        snF = pw.tile([128, T], bf16, tag="snF")
        nc.sync.dma_start(snF[:], sin2m.ap()[:])

        # persistent activations
        kT_sb = pbig.tile([128, T], bf16, tag="kT")
        V_sb = pbig.tile([128, NST, D], bf16, tag="V")
        rkS = pbig.tile([128, NST], f32, tag="rkS")

        def load_xt(tci):
            # full-width tile for steady-state chunks (fewer instructions)
            xtf = pxt.tile([128, NCC, TC], bf16, tag="xtf", name="xtf",
                           bufs=2)
            t0 = tci * TC
            for g0 in range(0, NCC, 4):
                nc.sync.dma_start(xtf[:, g0:g0 + 4, :],
                                  xT_re[:, g0:g0 + 4, t0:t0 + TC])
            return xtf

        xt_next = xt0

        fillers = []   # deferred PE-filler closures (out-proj of chunk i-1)
        yT_prev = None

        def flush_filler(n=1):
            for _ in range(min(n, len(fillers))):
                fillers.pop(0)()

        for tci in range(NTC):
            t0 = tci * TC
            r0, recs = plans[tci]

            # ================= phase 1: projections for chunk tci =========
            xt = xt_next
            cs = csF[:, t0:t0 + TC]
            sn = snF[:, t0:t0 + TC]
            if tci + 1 < NTC:
                xt_next = load_xt(tci + 1)
            if tci == 0:
                nc.scalar.dma_start(
                    wp_sb[:], wpT.ap().rearrange("(qc p) c -> p qc c", p=128))

            halves = isinstance(xt, list)
            pending = []   # deferred rope/norm chains

            def flush_pending(n=None):
                k = len(pending) if n is None else min(n, len(pending))
                for _ in range(k):
                    pending.pop(0)()

            def xsub(cc, c0, w):
                # chunk cols [c0, c0+w) of contraction chunk cc (w <= HTC)
                if halves:
                    return xt[c0 // HTC][:, cc, c0 % HTC:c0 % HTC + w]
                return xt[:, cc, c0:c0 + w]

            qT_sb = [pqt.tile([128, TC], bf16, tag=f"qT{m}", name=f"qT{m}")
                     for m in range(REP)]
            yT_sb = [pyt.tile([128, TC], bf16, tag=f"yT{m}", name=f"yT{m}")
                     for m in range(REP)]

            def emit_stream(si, kind, m):
                acc = psAcc.tile([128, TC], f32, tag="acc", name="acc")
                for hh in range(2 if halves else 1):
                    for cc in range(NCC):
                        if kind == "q":
                            lhsT = wq_sb[:, cc, m * D:(m + 1) * D]
                        else:
                            lhsT = wk_sb[:, cc, :]
                        if halves:
                            nc.tensor.matmul(
                                acc[:, hh * HTC:(hh + 1) * HTC], lhsT,
                                xt[hh][:, cc, :],
                                start=(cc == 0 and hh == 0),
                                stop=(cc == NCC - 1 and hh == 1))
                        else:
                            nc.tensor.matmul(
                                acc[:], lhsT, xt[:, cc, :],
                                start=(cc == 0), stop=(cc == NCC - 1))

                raw = ptmp.tile([128, TC], bf16, tag="t", name="raw")
                nc.vector.tensor_copy(raw[:], acc[:])
                sq = ptmp.tile([128, TC], bf16, tag="t", name="sq")
                nc.vector.tensor_mul(sq[:], raw[:], raw[:])
                # sum-of-squares via tiny column matmuls
                ssc = 8 + 4 * (si % 2)
                for j in range(NSUB):
                    nc.tensor.matmul(small[:, ssc + j:ssc + j + 1],
                                     sq[:, j * 128:(j + 1) * 128],
                                     ones_sb[:], start=True, stop=True)
                lncol = prow.tile([128, NSUB], f32r, tag="ln", name="lncol")
                nc.scalar.activation(lncol[:], small[:, ssc:ssc + NSUB],
                                     AF.Ln, scale=1.0 / D,
                                     bias=eps_col[:, 0:1])

                if kind == "k":
                    nc.scalar.activation(rkS[:, NSUB * tci:NSUB * (tci + 1)],
                                         lncol[:], AF.Exp, scale=-0.5)
                    rbc = None
                else:
                    rcol = prow.tile([128, NSUB], f32, tag="rc", name="rcol")
                    nc.scalar.activation(rcol[:], lncol[:], AF.Exp, scale=-0.5)
                    rows = psSC.tile([1, TC], f32, tag="sc", name="rows")
                    for j in range(NSUB):
                        nc.tensor.transpose(rows[0:1, j * 128:(j + 1) * 128],
                                            rcol[:, j:j + 1], eyeF_sb[:])
                    rrow = prow.tile([1, TC], bf16, tag="rr", name="rrow")
                    nc.scalar.copy(rrow[:], rows[:])
                    rbc = pbc.tile([128, TC], bf16, tag="bc", name="rbc")
                    nc.gpsimd.partition_broadcast(rbc[:], rrow[0:1, :])

                def rope(kind=kind, m=m, raw=raw, rbc=rbc):
                    qsw = ptmp.tile([128, TC], bf16, tag="t", name="qsw")
                    nc.sync.dma_start(qsw[0:64, :], raw[64:128, :])
                    nc.sync.dma_start(qsw[64:128, :], raw[0:64, :])
                    ta = ptmp.tile([128, TC], bf16, tag="t", name="ta")
                    nc.vector.tensor_mul(ta[:], raw[:], cs[:])
                    tb = ptmp.tile([128, TC], bf16, tag="t", name="tb")
                    nc.vector.tensor_mul(tb[:], qsw[:], sn[:])
                    if kind == "k":
                        nc.vector.tensor_add(kT_sb[:, t0:t0 + TC],
                                             ta[:], tb[:])
                    else:
                        qro = ptmp.tile([128, TC], bf16, tag="t", name="qro")
                        nc.vector.tensor_add(qro[:], ta[:], tb[:])
                        nc.vector.tensor_mul(qT_sb[m][:], qro[:], rbc[:])
                pending.append(rope)
                if len(pending) > 1:
                    flush_pending(1)
                flush_filler(1)

            def emit_v():
                # v in natural [s, D] layout (+ ve gate per subtile)
                for j in range(NSUB):
                    st = NSUB * tci + j
                    zg = small[:, 16 + 4 * (tci % 2) + j:
                               17 + 4 * (tci % 2) + j]
                    nc.tensor.matmul(zg,
                                     xsub(0, j * 128, 128)[0:VE_GATE_CH, :],
                                     wg_sb[:], start=True, stop=True)
                    ve_t = pve.tile([128, D], bf16, tag="ve", name="ve_t")
                    nc.sync.dma_start(ve_t[:],
                                      veN.ap()[st * 128:(st + 1) * 128, :])
                    vslot = small[:, 384:512]
                    for cc in range(NCC):
                        nc.tensor.matmul(vslot, xsub(cc, j * 128, 128),
                                         wv_sb[:, cc, :],
                                         start=(cc == 0), stop=(cc == NCC - 1))
                    ez = prow.tile([128, 1], f32r, tag="gtmp", name="ez")
                    nc.scalar.activation(ez[:], zg, AF.Exp, scale=-1.0)
                    ez1 = prow.tile([128, 1], f32r, tag="gtmp", name="ez1")
                    nc.vector.tensor_scalar_add(ez1[:], ez[:], 1.0)
                    gcol = prow.tile([128, 1], f32, tag="gc", name="gcol")
                    nc.vector.reciprocal(gcol[:], ez1[:])
                    vg = pve.tile([128, D], bf16, tag="vg", name="vg")
                    nc.vector.tensor_scalar_mul(vg[:], ve_t[:], gcol[:, 0:1])
                    nc.vector.tensor_add(V_sb[:, st, :], vslot, vg[:])
                    if j == 0:
                        flush_pending()
                    flush_filler(1)

            def emit_attn(h):
                yU = psY.tile([128, TC], f32, tag="yU", name="yU")
                dcol = small[:, 4 * (h % 2):4 * (h % 2) + NSUB]
                pends = []
                for idx, (st, v0, v1, cms, wms, dens, pieces) in                         enumerate(recs):
                    s0 = st * 128
                    nmm = len(cms) + len(wms)
                    sc = psSC.tile([128, TC], f32, tag="sc", name="sc")
                    nc.tensor.matmul(sc[:, v0:v1], kT_sb[:, s0:s0 + 128],
                                     qT_sb[h][:, v0:v1],
                                     start=True, stop=(nmm == 0))
                    for j in cms:
                        nmm -= 1
                        nc.tensor.matmul(sc[:, j * 128:(j + 1) * 128],
                                         eye_sb[:], masks_sb[:, 0, :],
                                         start=False, stop=(nmm == 0))
                    for j, bi in wms:
                        nmm -= 1
                        nc.tensor.matmul(sc[:, j * 128:(j + 1) * 128],
                                         eye_sb[:], masks_sb[:, 1 + bi, :],
                                         start=False, stop=(nmm == 0))
                    if len(pends) >= 2:
                        pends.pop(0)()
                    flush_filler(1)
                    P = pP.tile([128, TC], bf16, tag="P", name="P")
                    nc.scalar.activation(P[:, v0:v1], sc[:, v0:v1], AF.Exp,
                                         scale=rkS[:, st:st + 1])

                    def mk(P=P, st=st, dens=dens, pieces=pieces):
                        def go():
                            for j, fst, lst in dens:
                                nc.tensor.matmul(
                                    dcol[:, j:j + 1],
                                    P[:, j * 128:(j + 1) * 128], ones_sb[:],
                                    start=fst, stop=lst)
                            for c0, c1, fst, lst in pieces:
                                nc.tensor.matmul(
                                    yU[:, c0:c1], V_sb[:, st, :], P[:, c0:c1],
                                    start=fst, stop=lst)
                        return go
                    pends.append(mk())
                while pends:
                    pends.pop(0)()
                # normalize: yT = yU * (1/den) broadcast along partitions
                dinv = prow.tile([128, NSUB], f32, tag="di", name="dinv")
                nc.vector.reciprocal(dinv[:], dcol[:])
                rows = psSC.tile([1, TC], f32, tag="sc", name="rows")
                for j in range(NSUB):
                    nc.tensor.transpose(rows[0:1, j * 128:(j + 1) * 128],
                                        dinv[:, j:j + 1], eyeF_sb[:])
                drow = prow.tile([1, TC], f32, tag="dr", name="drow")
                nc.vector.tensor_copy(drow[:], rows[:])
                dbc = pdbc.tile([128, TC], f32, tag="dbc", name="dbc")
                nc.gpsimd.partition_broadcast(dbc[:], drow[0:1, :])
                nc.vector.tensor_mul(yT_sb[h][:], dbc[:], yU[:])
                flush_filler(1)

            emit_stream(0, "k", 0)
            emit_stream(1, "q", 0)
            emit_stream(2, "q", 1)
            emit_stream(3, "q", 2)
            emit_v()
            emit_attn(0)
            emit_attn(1)
            emit_attn(2)

            # ---- queue out-proj of this chunk as PE filler ----
            yT_prev = yT_sb

            def mk_proj(cc, yT=yT_sb, t0=t0):
                def go():
                    o = psO.tile([128, TC], f32, tag="o", name="o")
                    for m in range(REP):
                        nc.tensor.matmul(
                            o[:], wp_sb[:, m, cc * 128:(cc + 1) * 128],
                            yT[m][:], start=(m == 0), stop=(m == REP - 1))
                    ot = pout.tile([128, TC], bf16, tag="ot", name="ot")
                    if cc % 2 == 0:
                        nc.vector.tensor_copy(ot[:], o[:])
                    else:
                        nc.scalar.copy(ot[:], o[:])
                    nc.sync.dma_start(
                        outT.ap()[cc * 128:(cc + 1) * 128, t0:t0 + TC], ot[:])
                return go
            for cc in range(NCC):
                fillers.append(mk_proj(cc))
            if tci == NTC - 1:
                flush_filler(len(fillers))

    nc.compile()
    nc._mask_r0 = plans[0][0]
    return nc


def _prep_inputs(nc, window, x, ve, cos, sin, Wq, Wk, Wv, Wproj, Wg):
    """Build the 8 per-core input maps (host-side sharding + transposes)."""
    bf = ml_dtypes.bfloat16
    win_finite = 0 <= window < T
    cosT = np.ascontiguousarray(cos.reshape(T, D // 2).T)
    sinT = np.ascontiguousarray(sin.reshape(T, D // 2).T)
    cos2 = (_CS * np.concatenate([cosT, cosT], axis=0)).astype(bf)
    sin2m = (_CS * np.concatenate([sinT, -sinT], axis=0)).astype(bf)
    eye = np.eye(128, dtype=np.float32).astype(bf)
    ones = np.ones((128, 1), dtype=np.float32).astype(bf)

    p = np.arange(128)[:, None]
    c = np.arange(128)[None, :]
    r0 = nc._mask_r0
    cmask = np.where(c >= p, 0.0, _MASKVAL).astype(np.float32)
    w0 = np.where(c <= p - r0, 0.0, _MASKVAL).astype(np.float32)
    w1 = np.where(c <= p - (r0 - 128), 0.0, _MASKVAL).astype(np.float32)
    masks = np.concatenate([cmask, w0, w1], axis=0).astype(bf)

    xTb = [np.ascontiguousarray(x[b].T).astype(bf) for b in range(B)]

    in_maps = []
    for core in range(N_CORES):
        b, g = divmod(core, KV)
        sl_q = slice(g * QD, (g + 1) * QD)
        sl_d = slice(g * D, (g + 1) * D)
        in_maps.append({
            "xT": xTb[b],
            "wqT": np.ascontiguousarray(Wq[sl_q].T).astype(bf),
            "wkT": np.ascontiguousarray(Wk[sl_d].T).astype(bf),
            "wvT": np.ascontiguousarray(Wv[sl_d].T).astype(bf),
            "wpT": np.ascontiguousarray(Wproj[:, sl_q].T).astype(bf),
            "wg": np.ascontiguousarray(
                Wg[g].reshape(VE_GATE_CH, 1)).astype(bf),
            "veN": np.ascontiguousarray(3.0 * ve[b, :, sl_d]).astype(bf),
            "cos2": cos2, "sin2m": sin2m, "eye": eye, "onesI": ones,
            "masksI": masks,
        })
    return in_maps


def kernel(x, ve, cos, sin, Wq, Wk, Wv, Wproj, Wg, window, _trace=False):
    window = int(window)
    if window not in _CACHE:
        _CACHE[window] = _build(window)
    nc = _CACHE[window]

    in_maps = _prep_inputs(nc, window,
                           np.asarray(x, np.float32), np.asarray(ve, np.float32),
                           np.asarray(cos, np.float32), np.asarray(sin, np.float32),
                           np.asarray(Wq, np.float32), np.asarray(Wk, np.float32),
                           np.asarray(Wv, np.float32), np.asarray(Wproj, np.float32),
                           np.asarray(Wg, np.float32))

    res = run_bass_kernel_spmd(nc, in_maps, core_ids=list(range(N_CORES)),
                               trace=_trace)

    out = np.empty((B, T, C), dtype=np.float32)
    for b in range(B):
        acc = res.results[b * KV]["outT"].astype(np.float32)
        for g in range(1, KV):
            acc += res.results[b * KV + g]["outT"].astype(np.float32)
        out[b] = acc.T
    if _trace:
        kernel._last_trace = res
    return out
